# revision 25
# baseline (speedup 1.0000x reference)
"""Multi-head causal attention (B=2, S=2048, D=1024, H=16, Dh=64) on 8 TRN2
NeuronCores.

Sharding: tensor-parallel over heads - core c owns heads (2c, 2c+1) for both
batches. v2 design:
  - QKV projection as before (wT-stationary, x moving, 512-wide chunks).
  - Attention in transposed form (S^T = K Q^T). The two per-core heads run as
    two interleaved streams (h1 lagging h0 by one (b,qc) unit); head h's score
    matmuls use PE rows 64h..64h+63, so adjacent h0/h1 matmuls row-tile-pack
    and stream concurrently through the PE array.
  - Causal trimming: score fills, exp, and PV fills skip fully-masked columns
    (column offsets); only the 128-wide diagonal strip is masked via one
    [128,128] triangle mask on DVE.
  - PV is V_aug^T-stationary with a ones-column producing the softmax
    denominator for free. pv psum (unnormalized!) + denominator row are
    evacuated to bf16 and all-to-all'd; normalization happens on the
    DESTINATION core (reciprocal + broadcast-matmul + one DVE mult per af
    tile). This removes the rB matmuls / Ln/Exp / GpSimd mults from the
    critical path and keeps the GpSimd queue free for prompt cc triggers.
  - a2a(h0) fires while h1's trailing unit still computes; output projection
    h0-half matmuls overlap cc(h1).
"""
import ml_dtypes
import numpy as np

import concourse.bass as bass
import concourse.mybir as mybir
import concourse.tile as tile
from concourse.bass_utils import run_bass_kernel_spmd

F32 = mybir.dt.float32
BF16 = mybir.dt.bfloat16

B = 2
S = 2048
D = 1024
H = 16
DH = 64
N_CORES = 8
R = B * S          # 4096 global rows
RC = R // N_CORES  # 512 rows per core for the output projection

# ---------------------------------------------------------------------------
# BIR splitter: this toolchain's walrus rejects >1 sem-wait per instruction;
# move extra waits onto preceding same-engine nops (identical semantics).
def _split_waits(nc, maxw=1):
    for f in nc.m.functions:
        for bb in f.blocks:
            new_insts = []
            for ins in bb.instructions:
                si = ins.sync_info
                waits = list(si.on_wait) if si and si.on_wait else []
                if len(waits) > maxw:
                    carry, keep = waits[:-maxw], waits[-maxw:]
                    for j in range(0, len(carry), maxw):
                        new_insts.append(
                            mybir.InstNoOp(
                                name=f"{ins.name}-ws{j}",
                                engine=ins.engine,
                                sync_info=mybir.SyncInfo(
                                    on_wait=carry[j : j + maxw], on_update=[]
                                ),
                                bass_nofuse=True,
                            )
                        )
                    ins.sync_info = mybir.SyncInfo(
                        on_wait=keep,
                        on_update=list(si.on_update) if si.on_update else [],
                    )
                new_insts.append(ins)
            bb.instructions = new_insts


def _build():
    nc = bass.Bass()

    xT_d = nc.declare_dram_parameter("xT", [D, R], BF16, isOutput=False)
    wT_d = nc.declare_dram_parameter("wT", [D, 6 * DH], BF16, isOutput=False)
    woT_d = nc.declare_dram_parameter("woT", [D, D], BF16, isOutput=False)
    tri_d = nc.declare_dram_parameter("tri", [128, 128], BF16, isOutput=False)
    ident_d = nc.declare_dram_parameter("ident", [128, 128], BF16, isOutput=False)
    sel_d = nc.declare_dram_parameter("sel", [8, 512], BF16, isOutput=False)
    out_d = nc.declare_dram_parameter("out", [RC, D], F32, isOutput=True)

    # two sub-1MB a2a's (>=1MB flips mesh->RDH which hangs); both staged
    # up-front and triggered back-to-back
    a2a_in = [
        nc.dram_tensor(f"a2a_in{h}", [N_CORES, 65, RC], BF16) for h in range(2)
    ]
    a2a_out = [
        nc.dram_tensor(f"a2a_out{h}", [N_CORES, 65, RC], BF16) for h in range(2)
    ]

    NT = R // 512       # 8 column chunks of the projection
    NC_T = D // 128     # 8 contraction tiles
    NST = R // 128      # 32 s-tiles for V

    with tile.TileContext(nc) as tc:
      with nc.allow_low_precision(reason="bf16 attention pipeline"):
        with (
            tc.tile_pool(name="consts", bufs=1) as consts,
            tc.tile_pool(name="qk", bufs=1) as qk_pool,
            tc.tile_pool(name="vaug", bufs=1) as vaug_pool,
            tc.tile_pool(name="pvt", bufs=1) as pvt_pool,
            tc.tile_pool(name="work", bufs=6) as work,
            tc.tile_pool(name="wt", bufs=1) as wt_pool,
            tc.tile_pool(name="xs", bufs=3) as x_pool,
            tc.tile_pool(name="vt", bufs=1) as vt_pool,
            tc.tile_pool(name="wo", bufs=1) as wo_pool,
            tc.tile_pool(name="af", bufs=1) as af_pool,
            tc.tile_pool(name="outp", bufs=2) as out_pool,
        ):
            tri = None
            ident = None
            qT = qk_pool.tile([128, R], BF16, tag="qT")
            kT = qk_pool.tile([128, R], BF16, tag="kT")
            v_augs = [
                vaug_pool.tile([128, 130], BF16, tag=f"va{st}", name=f"va{st}")
                for st in range(NST)
            ]
            pvT = [
                pvt_pool.tile([65, R], BF16, tag=f"pvT{h}", name=f"pvT{h}")
                for h in range(2)
            ]
            # selector for recip broadcast: sel[s, 128t+p] = (s == 2t + p//64)
            sel = consts.tile([8, 512], BF16, tag="sel")
            nc.gpsimd.dma_start(out=sel, in_=sel_d[:, :])

            with (
                tc.tile_pool(name="psum_qkv", bufs=2, space="PSUM") as psum_qkv,
                tc.tile_pool(name="psum_pv", bufs=1, space="PSUM") as psum_pv,
                tc.tile_pool(name="psum_s", bufs=2, space="PSUM") as psum_s,
            ):
                wts = []
                for ct in range(NC_T):
                    wt = wt_pool.tile([128, 6 * DH], BF16, tag=f"wt{ct}")
                    nc.sync.dma_start(
                        out=wt, in_=wT_d[128 * ct : 128 * (ct + 1), :]
                    )
                    wts.append(wt)
                vT = vt_pool.tile([128, R], BF16, tag="vT")

                def emit_qkv_chunk(n):
                    nonlocal tri, ident
                    xts = []
                    for ct in range(NC_T):
                        xt = x_pool.tile(
                            [128, 512], BF16, tag=f"x{ct}", name=f"x{ct}_{n}"
                        )
                        nc.sync.dma_start(
                            out=xt,
                            in_=xT_d[128 * ct : 128 * (ct + 1), 512 * n : 512 * (n + 1)],
                        )
                        xts.append(xt)
                    if n == 1:
                        # consts deferred out of the critical first DMA wave
                        ident = consts.tile([128, 128], BF16, tag="ident")
                        nc.sync.dma_start(out=ident, in_=ident_d[:, :])
                        tri = consts.tile([128, 128], BF16, tag="tri")
                        nc.sync.dma_start(out=tri, in_=tri_d[:, :])
                    for mi, dst in ((0, qT), (1, kT), (2, vT)):
                        ps = psum_qkv.tile(
                            [128, 512], F32, tag="qkvp", name=f"qkvp{n}_{mi}"
                        )
                        for ct in range(NC_T):
                            nc.tensor.matmul(
                                ps,
                                lhsT=wts[ct][:, 128 * mi : 128 * (mi + 1)],
                                rhs=xts[ct],
                                start=(ct == 0),
                                stop=(ct == NC_T - 1),
                            )
                        nc.vector.tensor_copy(dst[:, 512 * n : 512 * (n + 1)], ps)

                def emit_va(st):
                    pt = psum_qkv.tile([128, 128], BF16, tag="qkvp", name=f"vtp{st}")
                    nc.tensor.transpose(
                        pt, vT[:, 128 * st : 128 * (st + 1)], ident
                    )
                    va = v_augs[st]
                    nc.vector.memset(va[:, 64:65], 1.0)
                    nc.vector.memset(va[:, 129:130], 1.0)
                    nc.vector.tensor_copy(va[:, 0:64], pt[:, 0:64])
                    nc.vector.tensor_copy(va[:, 65:129], pt[:, 64:128])

                # ---- batch-0 projection ------------------------------------
                for n in range(4):
                    emit_qkv_chunk(n)
                for st in range(16):
                    emit_va(st)

                # ---- attention: two staggered head streams; batch-1 QKV
                # chunks + V-transposes injected as PE filler ----------------
                G = {h: [] for h in (0, 1)}
                for h in (0, 1):
                    for b in range(B):
                        for qc in range(4):
                            for g in range(2 * qc + 2):
                                G[h].append((b, qc, g))
                n_g = len(G[0])  # 40
                LAG = 8

                pv_cur = {0: None, 1: None}

                def step(h, idx):
                    b, qc, g = G[h][idx]
                    hb = 64 * h
                    q0 = 2048 * b + 512 * qc
                    nkt = 4 * qc + 4
                    if g == 0:
                        pv_cur[h] = psum_pv.tile(
                            [65, 512], F32, tag=f"pv{h}", name=f"pv{h}_{idx}"
                        )
                    pv = pv_cur[h]
                    sp = psum_s.tile([128, 1024], F32, tag="sp")
                    offs = []
                    for half in (0, 1):
                        kt = 2 * g + half
                        m = kt - 4 * qc
                        off = 128 * m if m > 0 else 0
                        offs.append(off)
                        k0 = 2048 * b + 128 * kt
                        nc.tensor.matmul(
                            sp[:, 512 * half + off : 512 * (half + 1)],
                            lhsT=kT[hb : hb + 64, k0 : k0 + 128],
                            rhs=qT[hb : hb + 64, q0 + off : q0 + 512],
                            start=True,
                            stop=True,
                        )
                    e2 = work.tile([128, 1024], BF16, tag="expS", name=f"e{h}_{idx}")
                    off_a, off_b = offs
                    if 2 * g - 4 * qc >= 0:
                        # diagonal group: two exps so the unwritten psum strip
                        # [512+off_a, 512+off_b) is never read
                        nc.scalar.activation(
                            e2[:, off_a:512], sp[:, off_a:512],
                            mybir.ActivationFunctionType.Exp,
                            scale=0.125,
                        )
                        nc.scalar.activation(
                            e2[:, 512 + off_b : 1024], sp[:, 512 + off_b : 1024],
                            mybir.ActivationFunctionType.Exp,
                            scale=0.125,
                        )
                    else:
                        nc.scalar.activation(
                            e2[:, off_a:1024], sp[:, off_a:1024],
                            mybir.ActivationFunctionType.Exp,
                            scale=0.125,
                        )
                    for half in (0, 1):
                        m = 2 * g + half - 4 * qc
                        if m >= 0:
                            c0 = 512 * half + 128 * m
                            nc.vector.tensor_mul(
                                e2[:, c0 : c0 + 128], e2[:, c0 : c0 + 128], tri
                            )
                    for half in (0, 1):
                        kt = 2 * g + half
                        off = offs[half]
                        nc.tensor.matmul(
                            pv[:, off:512],
                            lhsT=v_augs[16 * b + kt][:, 65 * h : 65 * h + 65],
                            rhs=e2[:, 512 * half + off : 512 * (half + 1)],
                            start=(kt == 0),
                            stop=(kt == nkt - 1),
                        )
                    if g == nkt // 2 - 1:
                        nc.vector.tensor_copy(
                            pvT[h][:, q0 : q0 + 512], pv
                        )

                inject_qkv = {6: 4, 12: 5, 18: 6, 24: 7}
                inject_va = {26: (16, 20), 28: (20, 24), 30: (24, 28), 32: (28, 32)}
                gcount = 0

                def filler():
                    if gcount in inject_qkv:
                        emit_qkv_chunk(inject_qkv[gcount])
                    if gcount in inject_va:
                        lo, hi = inject_va[gcount]
                        for st in range(lo, hi):
                            emit_va(st)

                for i in range(LAG):
                    step(0, i)
                    gcount += 1
                    filler()
                for i in range(LAG, n_g):
                    step(0, i)
                    gcount += 1
                    filler()
                    step(1, i - LAG)
                    gcount += 1
                    filler()
                for i in range(n_g - LAG, n_g):
                    step(1, i)

                # w_o loads overlap the attention tail
                wos = []
                for dt in range(NC_T):
                    wo = wo_pool.tile([128, D], BF16, tag=f"wo{dt}")
                    nc.sync.dma_start(
                        out=wo, in_=woT_d[128 * dt : 128 * (dt + 1), :]
                    )
                    wos.append(wo)

                # ---- stage both payloads, then trigger both a2a's ----------
                for j in range(N_CORES):
                    nc.sync.dma_start(
                        out=a2a_in[0][j : j + 1, :, :],
                        in_=pvT[0][:, 512 * j : 512 * (j + 1)],
                    )
                    nc.gpsimd.dma_start(
                        out=a2a_in[1][j : j + 1, :, :],
                        in_=pvT[1][:, 512 * j : 512 * (j + 1)],
                    )
                for h in range(2):
                    nc.gpsimd.collective_compute(
                        "AllToAll",
                        mybir.AluOpType.bypass,
                        ins=[a2a_in[h][:]],
                        outs=[a2a_out[h][:]],
                        replica_groups=[list(range(N_CORES))],
                    )

            # ---- destination-side gather + normalize ---------------------
            afs = {}
            engs = [nc.sync, nc.gpsimd, nc.scalar]
            with tc.tile_pool(name="psum_bc", bufs=2, space="PSUM") as psum_bc:
                for h in range(2):
                    dn = af_pool.tile([8, RC], BF16, tag=f"dn{h}")
                    nc.gpsimd.dma_start(
                        out=dn, in_=a2a_out[h][:, 64:65, :]
                    )
                    rc = af_pool.tile([8, RC], F32, tag=f"rc{h}")
                    nc.vector.reciprocal(rc, dn)
                    rcb = af_pool.tile([8, RC], BF16, tag=f"rcb{h}")
                    nc.vector.tensor_copy(rcb, rc)
                    for t in range(4):
                        af = af_pool.tile(
                            [128, RC], BF16, tag=f"af{h}_{t}", name=f"af{h}_{t}"
                        )
                        engs[(2 * t) % 3].dma_start(
                            out=af[0:64, :],
                            in_=a2a_out[h][2 * t : 2 * t + 1, 0:64, :],
                        )
                        engs[(2 * t + 1) % 3].dma_start(
                            out=af[64:128, :],
                            in_=a2a_out[h][2 * t + 1 : 2 * t + 2, 0:64, :],
                        )
                        bc = psum_bc.tile([128, RC], F32, tag="bc")
                        nc.tensor.matmul(
                            bc,
                            lhsT=sel[:, 128 * t : 128 * (t + 1)],
                            rhs=rcb,
                            start=True,
                            stop=True,
                        )
                        nc.vector.tensor_mul(af, af, bc)
                        afs[(h, t)] = af

            # ---- output projection --------------------------------------
            with tc.tile_pool(name="psum_o", bufs=2, space="PSUM") as psum_o:
               for stile in range(RC // 128):
                  ot = out_pool.tile([128, D], F32, tag="out")
                  for dc in range(2):
                      po = psum_o.tile([128, 512], F32, tag="po")
                      for hh in range(2):
                          for t in range(4):
                              nc.tensor.matmul(
                                  po,
                                  lhsT=afs[(hh, t)][:, 128 * stile : 128 * (stile + 1)],
                                  rhs=wos[4 * hh + t][:, 512 * dc : 512 * (dc + 1)],
                                  start=(hh == 0 and t == 0),
                                  stop=(hh == 1 and t == 3),
                              )
                      nc.vector.tensor_copy(ot[:, 512 * dc : 512 * (dc + 1)], po)
                  nc.sync.dma_start(
                      out=out_d[128 * stile : 128 * (stile + 1), :], in_=ot
                  )

    _split_waits(nc, maxw=1)
    return nc


def _install_ntff_shim():
    """Register the NTFF profile hook that this image's `antenv` lacks.

    bass_utils reads `antenv.axon_hooks.get_axon_ntff_profile_hook()` when
    trace=True under axon; provide the module via sys.modules and wire the
    ctypes hook against the axon PJRT .so (same ABI trn_boot uses).
    """
    import sys
    import types
    import ctypes
    import contextlib

    if "antenv.axon_hooks" in sys.modules:
        return
    so_path = "/opt/axon/libaxon_pjrt.so"
    try:
        lib = ctypes.CDLL(so_path)
    except OSError:
        return
    if not hasattr(lib, "axon_start_nrt_profile"):
        return
    lib.axon_start_nrt_profile.argtypes = [
        ctypes.POINTER(ctypes.c_int64),
        ctypes.c_size_t,
    ]
    lib.axon_start_nrt_profile.restype = ctypes.c_int64
    lib.axon_stop_nrt_profile.argtypes = [ctypes.c_char_p]
    lib.axon_stop_nrt_profile.restype = ctypes.c_int64

    @contextlib.contextmanager
    def _hook(output_dir, device_ids):
        import jax

        jax.devices()
        if device_ids:
            ids = (ctypes.c_int64 * len(device_ids))(*device_ids)
            rc = lib.axon_start_nrt_profile(ids, len(device_ids))
        else:
            rc = lib.axon_start_nrt_profile(None, 0)
        if rc != 0:
            raise RuntimeError(f"axon_start_nrt_profile rc={rc}")
        try:
            yield
        finally:
            n = lib.axon_stop_nrt_profile(str(output_dir).encode())
            print(f"ntff profile: {n} file(s) written to {output_dir}")

    mod = types.ModuleType("antenv.axon_hooks")
    mod.get_axon_ntff_profile_hook = lambda: _hook
    mod.set_axon_ntff_profile_hook = lambda h: None
    sys.modules["antenv.axon_hooks"] = mod


_nc_cache = None


def _get_nc():
    global _nc_cache
    if _nc_cache is None:
        _nc_cache = _build()
    return _nc_cache


def _prep_inputs(x, w_qkv, w_o):
    x = np.asarray(x, dtype=np.float32)
    w_qkv = np.asarray(w_qkv, dtype=np.float32)
    w_o = np.asarray(w_o, dtype=np.float32)

    bf = ml_dtypes.bfloat16
    xT = np.ascontiguousarray(x.reshape(R, D).T.astype(bf))   # [D, R]
    woT_full = w_o.T  # [d, d'] contraction rows
    # row order matching af tiles: block dt = 4h+t, row r ->
    # original dim 256t + 128*(r//64) + 64h + (r%64)
    order = np.empty(D, dtype=np.int64)
    for h in range(2):
        for t in range(4):
            for r in range(128):
                order[128 * (4 * h + t) + r] = (
                    256 * t + 128 * (r // 64) + 64 * h + (r % 64)
                )
    woT = np.ascontiguousarray(woT_full[order].astype(bf))    # [D, D]

    w_q = w_qkv[0:D]
    w_k = w_qkv[D : 2 * D]
    w_v = w_qkv[2 * D : 3 * D]

    kk = np.arange(128)[:, None]
    qq = np.arange(128)[None, :]
    tri = (qq >= kk).astype(bf)

    ident = np.eye(128, dtype=bf)

    sel = np.zeros((8, 512), bf)
    for t in range(4):
        for p in range(128):
            sel[2 * t + p // 64, 128 * t + p] = 1

    in_maps = []
    for c in range(N_CORES):
        h0, h1 = 2 * c, 2 * c + 1
        cols = []
        for w in (w_q, w_k, w_v):
            cols.append(w[DH * h0 : DH * h0 + DH])
            cols.append(w[DH * h1 : DH * h1 + DH])
        # [6*DH, D] rows: q_h0,q_h1,k_h0,k_h1,v_h0,v_h1 -> transpose to [D, 6*DH]
        w_slice = np.concatenate(cols, axis=0)
        wT = np.ascontiguousarray(w_slice.T.astype(bf))
        in_maps.append(
            {
                "xT": xT,
                "wT": wT,
                "woT": woT,
                "tri": tri,
                "ident": ident,
                "sel": sel,
            }
        )
    return in_maps


def kernel(x, w_qkv, w_o, _trace=False):
    if _trace:
        _install_ntff_shim()
    nc = _get_nc()
    in_maps = _prep_inputs(x, w_qkv, w_o)
    res = run_bass_kernel_spmd(
        nc, in_maps, list(range(N_CORES)), trace=_trace
    )
    out = np.concatenate(
        [res.results[c]["out"] for c in range(N_CORES)], axis=0
    )  # [R, D]
    out = out.reshape(B, S, D)
    if _trace:
        kernel.last_exec_time_ns = res.exec_time_ns
        kernel.last_results = res
    return out
